# revision 1
# baseline (speedup 1.0000x reference)
"""Bilinear RoI pooling (grid_sample style) on 8 Trainium2 NeuronCores.

Strategy (data-parallel over boxes, per sharding hint):
  - The sampling grid is axis-aligned (theta has zero off-diagonals), so the
    kernel is a pure gather + weighted-sum. All coordinate/index/weight math
    is done host-side in numpy; the device kernel is gather + matmul + store.
  - feats [512, 64, 256] f32 becomes a host-built fp16 table T [H*W+1, 2C]
    with T[i] = [feats_row(i), feats_row(i+W)] (row-pair interleave, zeros
    where i+W is the y-overflow — that corner's bilinear weight is 0).
  - ONE 4KB SWDGE gather descriptor per sample point fetches all 4 bilinear
    corners: elem_step=2C, elem_size=4C reads T[r] and T[r+1] (r=y0*W+x0) =
    corners (y0,x0),(y1,x0),(y0,x1),(y1,x1). The x1 overflow at x0=W-1 has
    weight exactly 0; one pad row covers the r+1 window at the table end.
  - Descriptor j = point pt lands in gather partition pt%128: a block is 128
    points. Four accumulating fp16 matmuls per block, lhsT = diag weights
    [128, 128] (w_q * delta(p==n), densified on device from a compact
    [128, 4] per-block table), rhs = corner-q channels [128, 512], produce
    PSUM [128 pts, 512 ch]. PSUM -> SBUF -> DRAM as out3 [128, NBLK*C] fp16
    (per-partition contiguous runs); the host transposes to [B, C, 7, 7].
"""
import sys
import numpy as np

sys.path.insert(0, "/opt/trn_rl_repo")

OH = OW = 7
C, H, W = 512, 64, 256
HW = H * W
B_TOTAL = 2048
N_CORES = 8
B_LOCAL = B_TOTAL // N_CORES
NPTS = B_LOCAL * OH * OW          # 12544 points per core
NBLK = NPTS // 128                # 98 blocks of 128 points
CHUNK_BLK = 8                     # blocks per dma_gather (1024 descriptors)


def _build(nc, tc):
    from contextlib import ExitStack
    import concourse.mybir as mybir
    from concourse import bass

    f32 = mybir.dt.float32
    f16 = mybir.dt.float16
    i16 = mybir.dt.int16

    A = mybir.AluOpType
    feats_t = nc.dram_tensor("feats_t", [HW + 1, 2 * C], f16,
                             kind="ExternalInput")
    idxw_d = nc.dram_tensor("idxw", [128, NPTS // 16], i16,
                            kind="ExternalInput")
    wt_d = nc.dram_tensor("wt", [128, NBLK * 4], f16, kind="ExternalInput")
    mask_d = nc.dram_tensor("mask", [128, 128], f16, kind="ExternalInput")
    out_d = nc.dram_tensor("out3", [128, NBLK * C], f16,
                           kind="ExternalOutput")

    es = ExitStack()
    idx_s = es.enter_context(nc.sbuf_tensor("idx_s", [128, NPTS // 16], i16))
    wt_s = es.enter_context(nc.sbuf_tensor("wt_s", [128, NBLK, 4], f16))
    mask_s = es.enter_context(nc.sbuf_tensor("mask_s", [128, 128], f16))

    # gather source view: row stride 2C, window 4C (fetches rows r and r+1)
    src_ap = bass.AP(feats_t, 0, [[2 * C, HW], [1, 4 * C]])

    with tc.tile_pool(name="gpool", bufs=3) as gpool, \
         tc.tile_pool(name="wpool", bufs=2) as wpool, \
         tc.tile_pool(name="spool", bufs=4) as spool, \
         tc.tile_pool(name="psum", bufs=8, space="PSUM") as psum_pool:
        nc.sync.dma_start(out=idx_s[:, :], in_=idxw_d[:, :])
        nc.sync.dma_start(
            out=bass.AP(wt_s, 0, [[NBLK * 4, 128], [1, NBLK * 4]]),
            in_=wt_d[:, :])
        nc.sync.dma_start(out=mask_s[:, :], in_=mask_d[:, :])

        # ramped chunk sizes: small first chunks land fast so the
        # matmul/copy pipeline spins up without an engine-idle window
        # while the full-size chunks stream behind them.
        tail = [4, 4, 2, 2]
        body = NBLK - 6 - sum(tail)
        sizes = [2, 4] + [CHUNK_BLK] * (body // CHUNK_BLK) + tail
        rem = NBLK - sum(sizes)
        if rem:
            sizes.insert(2, rem)
        b0 = 0
        for ch, nb in enumerate(sizes):
            nidx = nb * 128
            Gt = gpool.tile([128, CHUNK_BLK, 4 * C], f16, name="Gt")
            nc.gpsimd.dma_gather(
                out_ap=Gt[:, :nb, :], in_ap=src_ap,
                idxs_ap=idx_s[:, b0 * 8: b0 * 8 + nidx // 16],
                num_idxs=nidx, num_idxs_reg=nidx, elem_size=4 * C,
                elem_step=2 * C, queue_num=ch % 4)
            # dense diag weights: wden[p, bi, q, n] =
            #   delta(p == n) * wt[p, (b0+bi)*4+q]
            wden = wpool.tile([128, CHUNK_BLK, 4, 128], f16, name="wden")
            nc.vector.tensor_tensor(
                out=wden[:, :nb, :, :],
                in0=bass.AP(mask_s, 0, [[128, 128], [0, nb * 4], [1, 128]]),
                in1=bass.AP(wt_s, b0 * 4, [[NBLK * 4, 128], [1, nb * 4],
                                           [0, 128]]),
                op=A.mult)
            stage = spool.tile([128, CHUNK_BLK, C], f16, name="stage")
            for bi in range(nb):
                blk = b0 + bi
                ps = psum_pool.tile([128, C], f32, name="ps")
                for q in range(4):
                    nc.tensor.matmul(
                        out=ps[:, :],
                        lhsT=wden[:, bi, q, :],
                        rhs=Gt[:, bi, q * C:(q + 1) * C],
                        start=(q == 0), stop=(q == 3))
                dst = stage[:, bi, :]
                if blk % 2 == 0:
                    nc.vector.tensor_copy(out=dst, in_=ps[:, :])
                else:
                    nc.scalar.activation(
                        out=dst, in_=ps[:, :],
                        func=mybir.ActivationFunctionType.Copy)
            nc.sync.dma_start(
                out=bass.AP(out_d, b0 * C, [[NBLK * C, 128], [1, nb * C]]),
                in_=stage[:, :nb, :])
            b0 += nb


def _host_prep(feats, boxes, Him, Wim):
    """Build the interleaved row-pair table and per-core gather indices /
    bilinear weights on the host."""
    ft = np.ascontiguousarray(
        feats.transpose(1, 2, 0).reshape(HW, C)).astype(np.float16)
    T = np.zeros((HW + 1, 2 * C), np.float16)
    T[:HW, :C] = ft
    T[:HW - W, C:] = ft[W:]          # row i+W; zero for y=63 (weight 0)

    B = boxes.shape[0]
    xc = boxes[:, 0].astype(np.float64)
    yc = boxes[:, 1].astype(np.float64)
    bw = boxes[:, 2].astype(np.float64)
    bh = boxes[:, 3].astype(np.float64)
    gl = np.linspace(-1.0, 1.0, 7)
    gx = gl[None, :] * ((bw - 1.0) / (Wim - 1.0))[:, None] \
        + ((2.0 * xc - Wim - 1.0) / (Wim - 1.0))[:, None]   # [B, 7]
    gy = gl[None, :] * ((bh - 1.0) / (Him - 1.0))[:, None] \
        + ((2.0 * yc - Him - 1.0) / (Him - 1.0))[:, None]
    ix = np.clip((gx + 1.0) * 0.5 * (W - 1), 0.0, W - 1.0)
    iy = np.clip((gy + 1.0) * 0.5 * (H - 1), 0.0, H - 1.0)
    x0 = np.floor(ix)
    y0 = np.floor(iy)
    wx = (ix - x0).astype(np.float32)                        # [B, 7]
    wy = (iy - y0).astype(np.float32)
    x0 = x0.astype(np.int32)
    y0 = y0.astype(np.int32)

    # per point pt = b*49 + oy*7 + ox ; one descriptor: idx = y0*W + x0
    idx = (y0[:, :, None] * W + x0[:, None, :]).reshape(B * 49)
    assert idx.max() <= HW - 1

    # corner order within the 4C window: (y0,x0),(y1,x0),(y0,x1),(y1,x1)
    uy, ux = 1.0 - wy, 1.0 - wx
    w4 = np.stack([
        (uy[:, :, None] * ux[:, None, :]),
        (wy[:, :, None] * ux[:, None, :]),
        (uy[:, :, None] * wx[:, None, :]),
        (wy[:, :, None] * wx[:, None, :]),
    ], axis=-1).reshape(B * 49, 4).astype(np.float32)
    return T, idx, w4


def _pack_core(idx, w4):
    """Wrap indices to [128, NPTS//16] int16 and compact per-point corner
    weights to [128, NBLK*4] fp16 (densified to diag lhsT on device)."""
    idxw = np.zeros((16, NPTS // 16), np.int16)
    j = np.arange(NPTS)
    idxw[j % 16, j // 16] = idx.astype(np.int16)
    idxw = np.tile(idxw, (8, 1))                  # replicate to 128 partitions

    # wt[p, blk*4+q] = w4[blk*128 + p, q]
    wv = np.transpose(w4.reshape(NBLK, 128, 4), (1, 0, 2))  # [p, blk, q]
    return idxw, np.ascontiguousarray(wv).reshape(128, NBLK * 4).astype(
        np.float16)


_CACHE = {}


def _mask_host():
    return np.eye(128, dtype=np.float16)


def _get_compiled():
    if "nc" in _CACHE:
        return _CACHE["nc"]
    import concourse.bacc as bacc
    import concourse.tile as tile
    nc = bacc.Bacc("TRN2", target_bir_lowering=False, debug=False,
                   num_swdge_queues=4)
    with tile.TileContext(nc) as tc:
        _build(nc, tc)
    nc.compile()
    _CACHE["nc"] = nc
    return nc


def _run(feats, boxes, Him, Wim, trace=False, tmpdir=None):
    from concourse.bass_utils import run_bass_kernel_spmd
    nc = _get_compiled()
    T, idx, w4 = _host_prep(feats, boxes, Him, Wim)
    mask = _mask_host()
    in_maps = []
    for i in range(N_CORES):
        s = slice(i * B_LOCAL * 49, (i + 1) * B_LOCAL * 49)
        idxw, wt = _pack_core(idx[s], w4[s])
        in_maps.append({"feats_t": T, "idxw": idxw, "wt": wt, "mask": mask})
    res = run_bass_kernel_spmd(nc, in_maps, list(range(N_CORES)),
                               trace=trace, tmpdir=tmpdir)
    outs = []
    for i in range(N_CORES):
        o = np.asarray(res.results[i]["out3"], np.float32)  # [128, NBLK*C]
        o = o.reshape(128, NBLK, C).transpose(1, 0, 2)      # -> [NPTS, C]
        outs.append(np.ascontiguousarray(
            o.reshape(B_LOCAL, 49, C).transpose(0, 2, 1)))
    out = np.concatenate(outs, 0).reshape(B_TOTAL, C, OH, OW)
    return out, res


def kernel(**inputs):
    feats = np.asarray(inputs["feats"], dtype=np.float32)
    boxes = np.asarray(inputs["boxes"], dtype=np.float32)
    Him = int(inputs["image_height"])
    Wim = int(inputs["image_width"])
    out, _ = _run(feats, boxes, Him, Wim, trace=False)
    return out



# revision 2
# speedup vs baseline: 1.2004x; 1.2004x over previous
"""Bilinear RoI pooling (grid_sample style) on 8 Trainium2 NeuronCores.

Strategy (data-parallel over boxes; all coordinate math host-side):
  - The affine grid is axis-aligned, so sampling is separable: each output
    point (b, oy, ox) is a 2x2 bilinear blend. Per box only ~6 distinct
    feature rows are touched by all 7 oy grid rows (box heights are 8-64 px
    on a stride-8 map), so the kernel gathers one descriptor per
    (box, x-column-run, distinct-y-row) instead of one per sample point:
    a 2C fp16 window [f(y,x0), f(y,x0+1)] from a row-major [HW+1, C] table
    (elem_step=C, elem_size=2C).
  - Descriptors are packed into tiles of <=128 descriptors covering <=18
    output columns (col = (b, ox); M = 7*18 = 126 output points <= 128).
    Columns of one box sharing the same x0 (narrow boxes) reuse one
    descriptor (multi-hot column weights).
  - Per tile, lhsT[k, (cl, oy)] factorizes as cmq[q][k, cl] * wyw[k, oy]
    (x-corner weight folded into the column mask), densified on-device by
    one DVE broadcast-multiply per (chunk, q). Two accumulating fp16
    matmuls per tile (q = x0 / x0+1 halves of the gathered window) produce
    PSUM [126 pts, C]; PSUM -> SBUF (alternating DVE/ACT) -> DRAM fp16.
  - Host transposes the per-tile point blocks back to [B, C, 7, 7].
"""
import sys
import numpy as np

sys.path.insert(0, "/opt/trn_rl_repo")

OH = OW = 7
C, H, W = 512, 64, 256
HW = H * W
B_TOTAL = 2048
N_CORES = 8
B_LOCAL = B_TOTAL // N_CORES
MAX_COLS = 18            # columns per tile -> M = 126
MAX_DESC = 128           # descriptors per tile (K partitions)
M = MAX_COLS * OH        # 126
CHUNK = 8                # tiles per dma_gather call / stage buffer


def _build(nc, tc, NT):
    from contextlib import ExitStack
    import concourse.mybir as mybir
    from concourse import bass

    f32 = mybir.dt.float32
    f16 = mybir.dt.float16
    i16 = mybir.dt.int16
    A = mybir.AluOpType

    feats_t = nc.dram_tensor("feats_t", [HW + 1, C], f16, kind="ExternalInput")
    idxw_d = nc.dram_tensor("idxw", [128, NT * 8], i16, kind="ExternalInput")
    cmq_d = nc.dram_tensor("cmq", [128, NT * 2 * MAX_COLS], f16,
                           kind="ExternalInput")
    wyw_d = nc.dram_tensor("wyw", [128, NT * OH], f16, kind="ExternalInput")
    out_d = nc.dram_tensor("out3", [128, NT * C], f16, kind="ExternalOutput")

    es = ExitStack()
    idx_s = es.enter_context(nc.sbuf_tensor("idx_s", [128, NT * 8], i16))
    cmq_s = es.enter_context(
        nc.sbuf_tensor("cmq_s", [128, NT, 2, MAX_COLS], f16))
    wyw_s = es.enter_context(nc.sbuf_tensor("wyw_s", [128, NT, OH], f16))

    # gather source: row pitch C, window 2C (rows i and i+1 = x0, x0+1)
    src_ap = bass.AP(feats_t, 0, [[C, HW], [1, 2 * C]])

    ncmq = NT * 2 * MAX_COLS
    nwyw = NT * OH

    with tc.tile_pool(name="gpool", bufs=3) as gpool, \
         tc.tile_pool(name="wpool", bufs=3) as wpool, \
         tc.tile_pool(name="spool", bufs=3) as spool, \
         tc.tile_pool(name="psum", bufs=8, space="PSUM") as psum_pool:
        nc.sync.dma_start(out=idx_s[:, :], in_=idxw_d[:, :])
        nc.sync.dma_start(
            out=bass.AP(cmq_s, 0, [[ncmq, 128], [1, ncmq]]), in_=cmq_d[:, :])
        nc.sync.dma_start(
            out=bass.AP(wyw_s, 0, [[nwyw, 128], [1, nwyw]]), in_=wyw_d[:, :])

        # ramped chunk sizes so the compute pipeline starts early
        sizes = [2, 4]
        body = NT - sum(sizes)
        sizes += [CHUNK] * (body // CHUNK)
        rem = NT - sum(sizes)
        if rem:
            sizes.append(rem)
        t0 = 0
        for ci, k in enumerate(sizes):
            Gt = gpool.tile([128, CHUNK, 2 * C], f16, name="Gt")
            nc.gpsimd.dma_gather(
                out_ap=Gt[:, :k, :], in_ap=src_ap,
                idxs_ap=idx_s[:, t0 * 8: (t0 + k) * 8],
                num_idxs=k * 128, num_idxs_reg=k * 128,
                elem_size=2 * C, elem_step=C, queue_num=ci % 4)
            # densify lhsT: wden[p, ti, q, cl*7+oy] = cmq[p,t,q,cl]*wyw[p,t,oy]
            wden = wpool.tile([128, CHUNK, 2, M], f16, name="wden")
            for q in range(2):
                nc.vector.tensor_tensor(
                    out=wden[:, :k, q, :],
                    in0=bass.AP(cmq_s, (t0 * 2 + q) * MAX_COLS,
                                [[NT * 2 * MAX_COLS, 128],
                                 [2 * MAX_COLS, k], [1, MAX_COLS], [0, OH]]),
                    in1=bass.AP(wyw_s, t0 * OH,
                                [[NT * OH, 128], [OH, k], [0, MAX_COLS],
                                 [1, OH]]),
                    op=A.mult)
            stage = spool.tile([128, CHUNK, C], f16, name="stage")
            for ti in range(k):
                t = t0 + ti
                ps = psum_pool.tile([128, C], f32, name="ps")
                for q in range(2):
                    nc.tensor.matmul(
                        out=ps[:M, :],
                        lhsT=wden[:, ti, q, :],
                        rhs=Gt[:, ti, q * C:(q + 1) * C],
                        start=(q == 0), stop=(q == 1))
                dst = stage[:M, ti, :]
                if t % 2 == 0:
                    nc.vector.tensor_copy(out=dst, in_=ps[:M, :])
                else:
                    nc.scalar.activation(
                        out=dst, in_=ps[:M, :],
                        func=mybir.ActivationFunctionType.Copy)
            nc.sync.dma_start(
                out=bass.AP(out_d, t0 * C, [[NT * C, M], [1, k * C]]),
                in_=stage[:M, :k, :])
            t0 += k


def _box_geometry(boxes, Him, Wim):
    xc, yc, bw, bh = [boxes[:, i].astype(np.float64) for i in range(4)]
    gl = np.linspace(-1.0, 1.0, 7)
    gx = gl[None, :] * ((bw - 1) / (Wim - 1))[:, None] \
        + ((2 * xc - Wim - 1) / (Wim - 1))[:, None]
    gy = gl[None, :] * ((bh - 1) / (Him - 1))[:, None] \
        + ((2 * yc - Him - 1) / (Him - 1))[:, None]
    ix = np.clip((gx + 1) * 0.5 * (W - 1), 0.0, W - 1.0)
    iy = np.clip((gy + 1) * 0.5 * (H - 1), 0.0, H - 1.0)
    x0 = np.floor(ix).astype(np.int64)
    y0 = np.floor(iy).astype(np.int64)
    wx = ix - x0
    wy = iy - y0
    y1 = np.minimum(y0 + 1, H - 1)
    return x0, wx, y0, y1, wy


def _prep_core(boxes, Him, Wim):
    """Tile metadata for one core's boxes. Returns (idx, cmq, wyw, meta)."""
    B = boxes.shape[0]
    x0, wx, y0, y1, wy = _box_geometry(boxes, Him, Wim)

    box_rows = []
    for b in range(B):
        acc = {}
        for oy in range(7):
            w0 = 1.0 - wy[b, oy]
            w1 = wy[b, oy]
            if w0 > 0:
                acc.setdefault(y0[b, oy], np.zeros(7))[oy] += w0
            if w1 > 0:
                acc.setdefault(y1[b, oy], np.zeros(7))[oy] += w1
        ys = sorted(acc.keys())
        box_rows.append((ys, np.stack([acc[y] for y in ys], 0)))

    # units: consecutive ox columns of one box sharing x0 (x0 is monotonic)
    units = []
    for b in range(B):
        ox = 0
        while ox < 7:
            a = x0[b, ox]
            oe = ox
            while oe + 1 < 7 and x0[b, oe + 1] == a:
                oe += 1
            units.append((b, int(a), ox, oe + 1))
            ox = oe + 1

    tiles, cur, cc, cd = [], [], 0, 0
    for u in units:
        b = u[0]
        ny = len(box_rows[b][0])
        ncol = u[3] - u[2]
        if cc + ncol > MAX_COLS or cd + ny > MAX_DESC:
            tiles.append(cur)
            cur, cc, cd = [], 0, 0
        cur.append(u)
        cc += ncol
        cd += ny
    if cur:
        tiles.append(cur)

    NT = len(tiles)
    idx = np.zeros((NT, MAX_DESC), np.int16)
    cmq = np.zeros((NT, 2, MAX_DESC, MAX_COLS), np.float16)
    wyw = np.zeros((NT, MAX_DESC, OH), np.float16)
    meta = []
    for t, tus in enumerate(tiles):
        k = 0
        cl = 0
        colmap = []
        for (b, a, os_, oe_) in tus:
            ys, wrow = box_rows[b]
            ny = len(ys)
            idx[t, k:k + ny] = (np.asarray(ys) * W + a).astype(np.int16)
            wyw[t, k:k + ny] = wrow.astype(np.float16)
            for ci, ox in enumerate(range(os_, oe_)):
                cmq[t, 0, k:k + ny, cl + ci] = np.float16(1.0 - wx[b, ox])
                cmq[t, 1, k:k + ny, cl + ci] = np.float16(wx[b, ox])
                colmap.append((b, ox))
            k += ny
            cl += oe_ - os_
        meta.append(colmap)
    return idx, cmq, wyw, meta


def _pack_core(idx, cmq, wyw, NT):
    """Pad to NT tiles and lay out device input tensors."""
    nt = idx.shape[0]
    # idxw: descriptor j of the whole stream at [j%16, j//16], tiled x8
    flat = np.zeros(NT * MAX_DESC, np.int16)
    flat[:nt * MAX_DESC] = idx.reshape(-1)
    idxw = np.zeros((16, NT * 8), np.int16)
    j = np.arange(NT * MAX_DESC)
    idxw[j % 16, j // 16] = flat
    idxw = np.tile(idxw, (8, 1))

    cmq_p = np.zeros((NT, 2, MAX_DESC, MAX_COLS), np.float16)
    cmq_p[:nt] = cmq
    wyw_p = np.zeros((NT, MAX_DESC, OH), np.float16)
    wyw_p[:nt] = wyw
    # device layout: [p=desc slot 128, t, q, cl] / [p, t, oy]
    cmq_dev = np.ascontiguousarray(
        cmq_p.transpose(2, 0, 1, 3)).reshape(128, NT * 2 * MAX_COLS)
    wyw_dev = np.ascontiguousarray(
        wyw_p.transpose(1, 0, 2)).reshape(128, NT * OH)
    return idxw, cmq_dev, wyw_dev


_CACHE = {}


def _get_compiled(NT):
    key = ("nc", NT)
    if key in _CACHE:
        return _CACHE[key]
    import concourse.bacc as bacc
    import concourse.tile as tile
    nc = bacc.Bacc("TRN2", target_bir_lowering=False, debug=False,
                   num_swdge_queues=4)
    with tile.TileContext(nc) as tc:
        _build(nc, tc, NT)
    nc.compile()
    _CACHE[key] = nc
    return nc


def _run(feats, boxes, Him, Wim, trace=False, tmpdir=None):
    from concourse.bass_utils import run_bass_kernel_spmd

    table = np.zeros((HW + 1, C), np.float16)
    table[:HW] = feats.transpose(1, 2, 0).reshape(HW, C).astype(np.float16)

    preps = []
    for i in range(N_CORES):
        preps.append(_prep_core(boxes[i * B_LOCAL:(i + 1) * B_LOCAL],
                                Him, Wim))
    NT = max(p[0].shape[0] for p in preps)
    nc = _get_compiled(NT)

    in_maps = []
    for i in range(N_CORES):
        idx, cmq, wyw, _ = preps[i]
        idxw, cmq_dev, wyw_dev = _pack_core(idx, cmq, wyw, NT)
        in_maps.append({"feats_t": table, "idxw": idxw,
                        "cmq": cmq_dev, "wyw": wyw_dev})
    res = run_bass_kernel_spmd(nc, in_maps, list(range(N_CORES)),
                               trace=trace, tmpdir=tmpdir)

    out = np.zeros((B_TOTAL, C, OH, OW), np.float32)
    for i in range(N_CORES):
        o = np.asarray(res.results[i]["out3"], np.float32)
        o = o.reshape(128, NT, C)
        meta = preps[i][3]
        for t, colmap in enumerate(meta):
            blk = o[:len(colmap) * OH, t, :]        # [ncols*7, C]
            for ci, (b, ox) in enumerate(colmap):
                out[i * B_LOCAL + b, :, :, ox] = blk[ci * 7:(ci + 1) * 7].T
    return out, res


def kernel(**inputs):
    feats = np.asarray(inputs["feats"], dtype=np.float32)
    boxes = np.asarray(inputs["boxes"], dtype=np.float32)
    Him = int(inputs["image_height"])
    Wim = int(inputs["image_width"])
    out, _ = _run(feats, boxes, Him, Wim, trace=False)
    return out
